# revision 18
# baseline (speedup 1.0000x reference)
# Trainium2 Bass kernel for nn_AttentionBlock (local 7x11 windowed attention).
#
# Strategy (data-parallel over batch, 4 batches/core on 8 cores):
#   - Rows are permuted to w-major order (n' = w*8 + h) so that the 7x11 local
#     attention window becomes band-structured over contiguous 128-key chunks.
#   - Per key-chunk kc (128 keys = 16 grid columns), only queries within +-5
#     grid columns can attend: a contiguous q-window of 168/208 entries.
#   - scores^T[k, q] computed directly (K=32 matmuls, 4 heads row-packed via
#     tile_position) so softmax-normalization/attn@v need NO transposes:
#       exp on ScalarE (scale folded), binary-mask multiply on GPSIMD,
#       per-(head,q) sums via ones-vector matmuls (col-packed M=1),
#       attn@v via col-packed M=32 matmuls accumulating over key chunks,
#       1/sums broadcast built with a gather-matrix matmul, applied on DVE.
#   - All matmuls run in bf16 (f32 accumulate). float32r would be more
#     accurate at the same speed, but its LDW expansion cannot carry even one
#     semaphore wait through this walrus, so it is unusable under Tile.
#   - 1/sums is computed as exp(-ln(sums)) on ScalarE: Ln+Exp share one ACT
#     table set (no table thrashing), custom-DVE recip ops don't compile here,
#     and nc.vector.reciprocal is ~6 cycles/element.
#   - b_proj is added on the host (it is zeros in this problem's setup).
import numpy as np
import ml_dtypes

B, H, WG, C, HEADS = 32, 8, 64, 256, 8
HK, WK = 7, 11
N = H * WG              # 512
HD = C // HEADS         # 32
SCALE = float(HD) ** -0.5
NCORES = 8
BPC = B // NCORES       # 4
WT = 16                 # key-chunk width (grid cols)
NKC = WG // WT          # 4
HALO = WK // 2          # 5

# n' = w*8 + h  ->  n = h*64 + w
PERM = np.array([(i % H) * WG + (i // H) for i in range(N)], dtype=np.int64)


def _kc_qwin(kc):
    c0 = max(0, WT * kc - HALO)
    c1 = min(WG, WT * kc + WT + HALO)
    return c0 * H, c1 * H


_NC_CACHE = {}

# walrus codegen rejects instructions whose sync-wait list exceeds the ISA
# struct's slot count (observed: Matmult >2 and f32r-Matmult/Ldweights >1
# fail with "Too many sync wait commands"). Tile does not split waits, so
# move the excess onto same-engine NoOps placed just before the instruction
# (FIFO order preserves the happens-before guarantee).
_WAIT_CAPS = {
    k: 1
    for k in (
        "InstMatmult", "InstLdweights", "InstActivation", "InstTensorTensor",
        "InstTensorCopy", "InstDMACopy", "InstDrain", "InstCustomDveAnt",
        "InstTensorScalarPtr", "InstMemset", "InstTensorReduce",
    )
}
_NOP_WAIT_CAP = 1


def _split_waits(nc):
    import concourse.mybir as mybir

    ctr = [0]
    for fn in nc.m.functions:
        for bb in fn.blocks:
            out = []
            for ins in bb.instructions:
                cap = _WAIT_CAPS.get(ins.__class__.__name__)
                si = getattr(ins, "sync_info", None)
                waits = list(si.on_wait) if si is not None else []
                if cap is not None and len(waits) > cap:
                    excess = waits[:-cap] if cap else waits
                    keep = waits[-cap:] if cap else []
                    while excess:
                        chunk = excess[:_NOP_WAIT_CAP]
                        excess = excess[_NOP_WAIT_CAP:]
                        w = mybir.InstEventSemaphore(
                            name=f"wsplit{ctr[0]}", ins=[], outs=[]
                        )
                        ctr[0] += 1
                        w.engine = ins.engine
                        w.sync_info = mybir.SyncInfo(
                            on_wait=chunk, on_update=[]
                        )
                        out.append(w)
                    ins.sync_info = mybir.SyncInfo(
                        on_wait=keep, on_update=list(si.on_update)
                    )
                out.append(ins)
            bb.instructions = out


def _build_nc(split_waits=True):
    key = ("nc", split_waits)
    if key in _NC_CACHE:
        return _NC_CACHE[key]
    import concourse.bass as bass
    import concourse.mybir as mybir
    import concourse.tile as tile

    f32 = mybir.dt.float32
    f32r = mybir.dt.float32r
    bf16 = mybir.dt.bfloat16
    EXP = mybir.ActivationFunctionType.Exp

    nc = bass.Bass("TRN2")

    xT = nc.dram_tensor("xT", [BPC, 2, 128, N], bf16, kind="ExternalInput")
    wqkT = nc.dram_tensor("wqkT", [2, 128, 512], bf16, kind="ExternalInput")
    wvT = nc.dram_tensor("wvT", [2, 128, 256], bf16, kind="ExternalInput")
    wpT = nc.dram_tensor("wpT", [2, 128, 256], bf16, kind="ExternalInput")
    m01T = {}
    for kc in range(NKC):
        qw0, qw1 = _kc_qwin(kc)
        m01T[kc] = nc.dram_tensor(
            f"m01T{kc}", [128, 4 * (qw1 - qw0)], bf16, kind="ExternalInput"
        )
    gsel = nc.dram_tensor("gsel", [128, 128], bf16, kind="ExternalInput")
    sumrow = nc.dram_tensor("sumrow", [1, 128], bf16, kind="ExternalInput")
    zrow = nc.dram_tensor("zrow", [1, 128], bf16, kind="ExternalInput")
    onesr = nc.dram_tensor("onesr", [1, 512], bf16, kind="ExternalInput")
    onesc = nc.dram_tensor("onesc", [128, 1], bf16, kind="ExternalInput")
    out = nc.dram_tensor("out", [BPC, N, C], f32, kind="ExternalOutput")

    with tile.TileContext(nc) as tc:
        import contextlib

        with contextlib.ExitStack() as ctx:
            singles = ctx.enter_context(tc.tile_pool(name="singles", bufs=1))
            sb = ctx.enter_context(tc.tile_pool(name="sb", bufs=2))
            ps = ctx.enter_context(tc.tile_pool(name="ps", bufs=2, space="PSUM"))

            # ---- load constants ----
            s_wqk = [singles.tile([128, 512], bf16, name=f"s_wqk{i}") for i in range(2)]
            s_wv = [singles.tile([128, 256], bf16, name=f"s_wv{i}") for i in range(2)]
            s_wp = [singles.tile([128, 256], bf16, name=f"s_wp{i}") for i in range(2)]
            for cc in range(2):
                nc.sync.dma_start(out=s_wqk[cc], in_=wqkT[cc])
                nc.sync.dma_start(out=s_wv[cc], in_=wvT[cc])
                nc.sync.dma_start(out=s_wp[cc], in_=wpT[cc])
            s_m01 = {}
            for kc in range(NKC):
                qw0, qw1 = _kc_qwin(kc)
                s_m01[kc] = singles.tile([128, 4 * (qw1 - qw0)], bf16, name=f"s_m01_{kc}")
                nc.sync.dma_start(out=s_m01[kc], in_=m01T[kc][:, :])
            s_gsel = singles.tile([128, 128], bf16)
            nc.sync.dma_start(out=s_gsel, in_=gsel[:, :])
            s_sumrow = singles.tile([1, 128], bf16)
            nc.sync.dma_start(out=s_sumrow, in_=sumrow[:, :])
            s_zrow = singles.tile([1, 128], bf16)
            nc.sync.dma_start(out=s_zrow, in_=zrow[:, :])
            s_onesr = singles.tile([1, 512], bf16)
            nc.sync.dma_start(out=s_onesr, in_=onesr[:, :])
            s_onesc = singles.tile([128, 1], bf16)
            nc.sync.dma_start(out=s_onesc, in_=onesc[:, :])

            for b in range(BPC):
                # ---- load xT (c-major) ----
                x_t = [sb.tile([128, N], bf16, tag="xT", bufs=4, name=f"x_t{i}") for i in range(2)]
                for cc in range(2):
                    nc.sync.dma_start(out=x_t[cc], in_=xT[b, cc])

                # ---- qk projection: qkT[f, n] for f in 0..512 (q: h0-7, k: h0-7)
                # psum layout: two [128,1024] tiles: fc pairs (0,1)=q, (2,3)=k
                s_qk = sb.tile([128, 2048], bf16, tag="qk", bufs=2)
                for pair in range(2):
                    p_qk = ps.tile([128, 1024], f32, tag="s", bufs=2)
                    for sub in range(2):
                        fc = pair * 2 + sub
                        for cc in range(2):
                            nc.tensor.matmul(
                                p_qk[:, sub * 512:(sub + 1) * 512],
                                lhsT=s_wqk[cc][:, fc * 128:(fc + 1) * 128],
                                rhs=x_t[cc][:, :],
                                start=(cc == 0),
                                stop=(cc == 1),
                            )
                    nc.vector.tensor_copy(
                        s_qk[:, pair * 1024:(pair + 1) * 1024], p_qk[:, :]
                    )

                # ---- v projection: v[n, c] natural, bf16, per key-chunk tile
                s_v = []
                for kcb in range(NKC):
                    p_v = ps.tile([128, 1024], f32, tag="s", bufs=2)
                    for cc in range(2):
                        nc.tensor.matmul(
                            p_v[:, 0:256],
                            lhsT=x_t[cc][:, kcb * 128:(kcb + 1) * 128],
                            rhs=s_wv[cc][:, :],
                            start=(cc == 0),
                            stop=(cc == 1),
                        )
                    sv = sb.tile([128, 256], bf16, tag="v", bufs=8)
                    nc.vector.tensor_copy(sv, p_v[:, 0:256])
                    s_v.append(sv)

                # ---- preclear accumulators ----
                # avT: [128 (4h x 32d), 512 q] per half; sums: rows {0,32,64,96}
                p_avT = []
                p_sums = []
                for half in range(2):
                    pa = ps.tile([128, 512], f32, tag="avT", bufs=2)
                    nc.tensor.matmul(
                        pa[:, :], lhsT=s_zrow[:, :], rhs=s_onesr[:, :],
                        start=True, stop=True, skip_group_check=True,
                    )
                    p_avT.append(pa)
                    pss = ps.tile([128, 512], f32, tag="sums", bufs=2)
                    nc.tensor.matmul(
                        pss[:, :], lhsT=s_sumrow[:, :], rhs=s_onesr[:, :],
                        start=True, stop=True, skip_group_check=True,
                    )
                    p_sums.append(pss)

                # ---- attention over key chunks ----
                # Concurrent row-tiled matmuls writing the same PSUM bank
                # crash the device, so scores go in 2-head groups with each
                # head's output slice filling a whole bank (512 f32).
                for kc in range(NKC):
                    qw0, qw1 = _kc_qwin(kc)
                    Wq = qw1 - qw0
                    for g in range(4):          # head group: heads 2g, 2g+1
                        half = g // 2
                        p_s = ps.tile([128, 1024], f32, tag="s", bufs=2)
                        for i in range(2):
                            h = 2 * g + i
                            j = h % 4           # row band within the f-chunk
                            koff = (2 + half) * 512 + kc * 128
                            nc.tensor.matmul(
                                p_s[:, i * 512: i * 512 + Wq],
                                lhsT=s_qk[32 * j:32 * j + 32, koff:koff + 128],
                                rhs=s_qk[32 * j:32 * j + 32,
                                         half * 512 + qw0: half * 512 + qw1],
                                start=True, stop=True,
                                tile_position=(32 * j, 0),
                            )
                        # exp (scale folded), PSUM->SBUF bf16
                        e_t = sb.tile([128, 2 * Wq], bf16, tag="eT", bufs=4)
                        nc.scalar.activation(
                            e_t.rearrange("p (j s) -> p j s", j=2),
                            p_s.rearrange("p (j s) -> p j s", j=2)[:, :, :Wq],
                            EXP, scale=SCALE,
                        )
                        # binary mask multiply (GPSIMD, frees DVE)
                        p_t = sb.tile([128, 2 * Wq], bf16, tag="pT", bufs=4)
                        nc.gpsimd.tensor_mul(p_t, e_t, s_m01[kc][:, :2 * Wq])
                        # per-(head, q) sums: ones-matmul, col-packed M=1
                        for i in range(2):
                            h = 2 * g + i
                            j = h % 4
                            nc.tensor.matmul(
                                p_sums[half][32 * j:32 * j + 1, qw0:qw1],
                                lhsT=s_onesc[:, :],
                                rhs=p_t[:, i * Wq:(i + 1) * Wq],
                                start=False, stop=(kc == NKC - 1),
                                tile_position=(0, 32 * j),
                                skip_group_check=True,
                            )
                        # attn @ v: col-packed M=32, accumulate over kc
                        for i in range(2):
                            h = 2 * g + i
                            j = h % 4
                            nc.tensor.matmul(
                                p_avT[half][32 * j:32 * j + 32, qw0:qw1],
                                lhsT=s_v[kc][:, h * 32:(h + 1) * 32],
                                rhs=p_t[:, i * Wq:(i + 1) * Wq],
                                start=False, stop=(kc == NKC - 1),
                                tile_position=(0, 32 * j),
                                skip_group_check=True,
                            )

                # ---- normalize: avT_n = avT * (1/sums) broadcast over d ----
                avT_sb = []
                for half in range(2):
                    # 1/s = exp(-ln(s)); Ln and Exp share one ACT table set
                    # (custom-DVE recip doesn't compile with this walrus, and
                    # ACT Reciprocal would thrash table sets against Exp).
                    lns = sb.tile([128, 512], f32, tag="lns", bufs=2)
                    nc.scalar.activation(
                        lns, p_sums[half][:, :],
                        mybir.ActivationFunctionType.Ln,
                    )
                    r_full = sb.tile([128, 512], bf16, tag="r", bufs=2)
                    nc.scalar.activation(
                        r_full, lns, EXP, scale=-1.0,
                    )
                    p_R = ps.tile([128, 1024], f32, tag="s", bufs=2)
                    nc.tensor.matmul(
                        p_R[:, 0:512],
                        lhsT=s_gsel[:, :],
                        rhs=r_full[:, :],
                        start=True, stop=True,
                    )
                    r_sb = sb.tile([128, 512], f32, tag="Rsb", bufs=2)
                    nc.any.tensor_copy(r_sb, p_R[:, 0:512])
                    av = sb.tile([128, 512], bf16, tag="av", bufs=3)
                    nc.vector.tensor_mul(av, r_sb, p_avT[half][:, :])
                    avT_sb.append(av)

                # ---- output projection (fp32r) + store ----
                for qc in range(4):
                    p_o = ps.tile([128, 1024], f32, tag="s", bufs=2)
                    for half in range(2):
                        nc.tensor.matmul(
                            p_o[:, 0:256],
                            lhsT=avT_sb[half][:, qc * 128:(qc + 1) * 128],
                            rhs=s_wp[half][:, :],
                            start=(half == 0), stop=(half == 1),
                        )
                    o_sb = sb.tile([128, 256], f32, tag="osb", bufs=3)
                    nc.any.tensor_copy(o_sb, p_o[:, 0:256])
                    nc.sync.dma_start(
                        out=out[b, qc * 128:(qc + 1) * 128, :], in_=o_sb
                    )

    if split_waits:
        _split_waits(nc)
    _NC_CACHE[key] = nc
    return nc


def _host_inputs(x, w_qkv, mask_np):
    """Build per-core input maps (host-side reshapes/permutes only)."""
    bf16 = ml_dtypes.bfloat16
    xp = np.ascontiguousarray(x[:, PERM, :])                      # [B, N, C]
    xTp = np.ascontiguousarray(np.transpose(xp, (0, 2, 1)))       # [B, C, N]
    xTp = xTp.reshape(B, 2, 128, N).astype(bf16)

    wqkT = np.ascontiguousarray(w_qkv[:512].T).reshape(2, 128, 512).astype(bf16)
    wvT = np.ascontiguousarray(w_qkv[512:].T).reshape(2, 128, 256).astype(bf16)

    m01p = (mask_np[PERM][:, PERM] == 0.0)
    m_tiles = {}
    for kc in range(NKC):
        qw0, qw1 = _kc_qwin(kc)
        t = m01p[qw0:qw1, 128 * kc:128 * kc + 128].T.astype(np.float32)  # [128, Wq]
        m_tiles[f"m01T{kc}"] = np.ascontiguousarray(
            np.concatenate([t] * 4, axis=1)
        ).astype(bf16)

    # gather/selection matrix: out-row m takes r from row 32*(m//32)
    gs = np.zeros((128, 128), dtype=np.float32)
    for m in range(128):
        gs[32 * (m // 32), m] = 1.0
    sr = np.ones((1, 128), dtype=np.float32)
    sr[0, [0, 32, 64, 96]] = 0.0

    base = {
        "wqkT": wqkT,
        "wvT": wvT,
        "gsel": gs.astype(bf16),
        "sumrow": sr.astype(bf16),
        "zrow": np.zeros((1, 128), dtype=bf16),
        "onesr": np.ones((1, 512), dtype=bf16),
        "onesc": np.ones((128, 1), dtype=bf16),
    }
    base.update(m_tiles)
    in_maps = []
    for core in range(NCORES):
        m = dict(base)
        m["xT"] = np.ascontiguousarray(xTp[core * BPC:(core + 1) * BPC])
        in_maps.append(m)
    return in_maps


def run_sharded(x, w_qkv, w_proj, b_proj, mask, trace=False):
    """Compile+run on 8 cores; returns (out_full, BassKernelResults)."""
    from concourse.bass_utils import run_bass_kernel_spmd

    x = np.asarray(x, dtype=np.float32)
    w_qkv = np.asarray(w_qkv, dtype=np.float32)
    w_proj = np.asarray(w_proj, dtype=np.float32)
    b_proj = np.asarray(b_proj, dtype=np.float32)
    mask_np = np.asarray(mask, dtype=np.float32).reshape(N, N)

    nc = _build_nc()
    in_maps = _host_inputs(x, w_qkv, mask_np)
    import ml_dtypes as _md
    wpT = np.ascontiguousarray(w_proj.T).reshape(2, 128, 256).astype(_md.bfloat16)
    for m in in_maps:
        m["wpT"] = wpT

    res = run_bass_kernel_spmd(nc, in_maps, core_ids=list(range(NCORES)), trace=trace)

    out_full = np.empty((B, N, C), dtype=np.float32)
    for core in range(NCORES):
        od = res.results[core]["out"]          # [BPC, N, C], permuted rows
        for bi in range(BPC):
            out_full[core * BPC + bi][PERM, :] = od[bi]
    out_full += b_proj[None, None, :]
    return out_full, res


def kernel(x, w_qkv, w_proj, b_proj, mask):
    out, _ = run_sharded(x, w_qkv, w_proj, b_proj, mask, trace=False)
    return out
